# revision 18
# baseline (speedup 1.0000x reference)
"""Causal self-attention Trainium2 kernel.

Reference computation (B=2, T=2048, EMB=1024, H=16 heads, D=64):
    qkv = x @ Wqkv + bqkv ; split q,k,v ; per-head causal softmax attention ;
    out = concat_heads @ Wout + bout

Sharding: 8 cores = data-parallel over batch (2) x tensor-parallel over
heads (4 heads/core).  Each core computes, for its (batch b, head shard m):
  - qkT = (x_b @ Wqk_m)^T  in d-major layout [512, 2048]  (+ bias)
  - v   = x_b @ Wv_m       in t-major layout (v bias folded into bout on host)
  - per head: S^T = k q^T (scale folded into exp), E = exp(S^T) restricted to
    the causal region, O'^T = [v | 1]^T E^T  (ones column yields softmax sums),
    normalize via reciprocal + K=1 matmul partition-broadcast
  - partial out = O_norm^T^T @ Wout_m  -> host sums the 4 TP partials per batch
Data path runs in bfloat16 (fp32 PSUM accumulation; softmax normalization in
fp32r): inputs are downcast host-side, halving DMA traffic, and every matmul
streams at the PE's 1 column/cycle fast path with no fp32r small-N penalty.
Score matmuls for causal-diagonal blocks stream only the unmasked columns.
"""

import sys

sys.path.insert(0, "/opt/trn_rl_repo")

import numpy as np

B, T, EMB = 2, 2048, 1024
H, D = 16, 64
N_CORES = 8
TP = 4  # head shards
HEADS_PER_CORE = H // TP  # 4
FSH = HEADS_PER_CORE * D  # 256 features per shard for each of q,k,v
P = 128
NG = T // 512  # 4 query groups of 512
NT = T // P  # 16 tiles of 128

_prog_cache = {}


def _build_program(
    repeat=1,
    skip_exp=False,
    skip_out_dma=False,
    reuse_x=False,
    skip_scores=False,
    skip_o=False,
    skip_proj=False,
    skip_ocopy=False,
):
    # skip_exp / skip_out_dma / reuse_x are timing-ablation knobs used only by
    # the local bench harness; the graded path always uses the defaults.
    import concourse.mybir as mybir
    import concourse.tile as tile
    from concourse import bacc

    f32 = mybir.dt.float32
    f32r = mybir.dt.float32r
    bf16 = mybir.dt.bfloat16
    AF = mybir.ActivationFunctionType
    OP = mybir.AluOpType

    nc = bacc.Bacc("TRN2", target_bir_lowering=False)

    xT_d = nc.dram_tensor("xT", (EMB, T), bf16, kind="ExternalInput")
    wqk_d = nc.dram_tensor("wqk", (EMB, 2 * FSH), bf16, kind="ExternalInput")
    wv_d = nc.dram_tensor("wv", (EMB, FSH), bf16, kind="ExternalInput")
    wout_d = nc.dram_tensor("wout", (FSH, EMB), bf16, kind="ExternalInput")
    bqk_d = nc.dram_tensor("bqk", (P, 4), f32, kind="ExternalInput")
    tri_d = nc.dram_tensor("trimask", (P, P), bf16, kind="ExternalInput")
    out_d = nc.dram_tensor("out", (T, EMB), bf16, kind="ExternalOutput")

    EK = EMB // P  # 8 contraction chunks
    xT_r = xT_d.rearrange("(o p) t -> p o t", p=P)

    with nc.allow_low_precision(
        reason="float32r tiles feed the PE fast path; fp32 accumulation in PSUM"
    ), tile.TileContext(nc) as tc:
        with (
            tc.tile_pool(name="consts", bufs=1) as consts,
            tc.tile_pool(name="qkt", bufs=1) as qkt_p,
            tc.tile_pool(name="vp", bufs=1) as v_p,
            tc.tile_pool(name="onorm", bufs=1) as onorm_p,
            tc.tile_pool(name="xp", bufs=4 if reuse_x else 2) as xp,
            tc.tile_pool(name="ep", bufs=3) as e_p,
            tc.tile_pool(name="bc", bufs=2) as bc_p,
            tc.tile_pool(name="outsb", bufs=5) as out_p,
            tc.tile_pool(name="psP", bufs=2, space="PSUM") as psP,
            tc.tile_pool(name="psS", bufs=1, space="PSUM") as psS,
            tc.tile_pool(name="psO", bufs=1, space="PSUM") as psO,
        ):
            # Startup-critical ordering: bias (tiny) + wqk f-tile 0 first, then
            # x chunk 0 is issued in the rep loop, then remaining weights are
            # staged so each arrives just before its first consumer.
            wqk_r = wqk_d.rearrange("(o p) f -> p o f", p=P)
            # one tile per f-slice so the first consumer only waits on its own DMA
            wqk_fs = [
                consts.tile([P, EK, P], bf16, name=f"wqk_f{f}") for f in range(4)
            ]
            wv_sb = consts.tile([P, EK, FSH], bf16)
            bqk_sb = consts.tile([P, 4], f32)
            tri_sb = consts.tile([P, P], bf16)
            wout_sb = consts.tile([P, 2, EMB], bf16)

            nc.sync.dma_start(bqk_sb[:], bqk_d[:])
            nc.sync.dma_start(wqk_fs[0][:], wqk_r[:, :, 0:P])

            def load_mid_consts():
                nc.sync.dma_start(wqk_fs[1][:], wqk_r[:, :, P : 2 * P])
                nc.sync.dma_start(wv_sb[:], wv_d.rearrange("(o p) f -> p o f", p=P))
                nc.sync.dma_start(wqk_fs[2][:], wqk_r[:, :, 2 * P : 3 * P])
                nc.sync.dma_start(wqk_fs[3][:], wqk_r[:, :, 3 * P : 4 * P])
                nc.sync.dma_start(tri_sb[:], tri_d[:])

            def load_wout():
                nc.sync.dma_start(
                    wout_sb[:], wout_d.rearrange("(o p) f -> p o f", p=P)
                )

            qkT = qkt_p.tile([P, 2, T], bf16)
            # per-head k tiles, zero-padded on the opposite 64-row half so the
            # score matmul can contract over all 128 partitions (the zero rows
            # cancel the other head packed in the shared q tile)
            kpad = qkt_p.tile([P, HEADS_PER_CORE, T], bf16)
            nc.vector.memset(kpad[:], 0.0)
            # per-head V blocks padded to 128 columns: cols 0-63 hold V, cols
            # 64-127 are all-ones so the O matmul (M=128, full-speed) emits the
            # softmax sums replicated on output partitions 64-127
            v_sb = v_p.tile([P, NT, HEADS_PER_CORE * P], bf16)
            v_ones_view = v_sb.rearrange("p t (h c) -> p t h c", c=P)[:, :, :, D:P]
            nc.vector.memset(v_ones_view, 1.0)

            x_chunks = {}

            for _rep in range(repeat):
                onorm = onorm_p.tile([P, 2, T], bf16)

                # ---------- chunked x load + deferred PE work ----------

                def load_x_chunk(g):
                    if reuse_x and g in x_chunks:
                        return
                    x_chunks[g] = xp.tile([P, EK, 512], bf16, name="xchunk")
                    nc.sync.dma_start(
                        x_chunks[g][:], xT_r[:, :, 512 * g : 512 * (g + 1)]
                    )

                def qk_chunk(f, g):
                    def run():
                        ps = psP.tile([P, 512], f32, tag="pp", name="ps_qk")
                        if skip_proj:
                            nc.tensor.matmul(
                                ps[:, 0:32],
                                wqk_fs[f][:, 0, :],
                                x_chunks[g][:, 0, 0:32],
                                start=True,
                                stop=True,
                            )
                        else:
                            for e in range(EK):
                                nc.tensor.matmul(
                                    ps[:],
                                    wqk_fs[f][:, e, :],
                                    x_chunks[g][:, e, :],
                                    start=(e == 0),
                                    stop=(e == EK - 1),
                                )
                        if f < 2:
                            nc.vector.tensor_scalar_add(
                                qkT[:, f, 512 * g : 512 * (g + 1)],
                                ps[:],
                                bqk_sb[:, f : f + 1],
                            )
                        else:
                            # k features of heads 2(f-2) / 2(f-2)+1 land on
                            # partition halves 0-63 / 64-127 — exactly the
                            # zero-padded halves their kpad slices use
                            h0 = 2 * (f - 2)
                            nc.vector.tensor_scalar_add(
                                kpad[0:64, h0, 512 * g : 512 * (g + 1)],
                                ps[0:64, :],
                                bqk_sb[0:64, f : f + 1],
                            )
                            nc.vector.tensor_scalar_add(
                                kpad[64:128, h0 + 1, 512 * g : 512 * (g + 1)],
                                ps[64:128, :],
                                bqk_sb[64:128, f : f + 1],
                            )
                    return run

                def v_chunk(t):
                    def run():
                        g, lt = divmod(t, 4)
                        ps = psP.tile([P, FSH], f32, tag="pp", name="ps_v")
                        if skip_proj:
                            nc.tensor.matmul(
                                ps[:, 0:32],
                                x_chunks[g][:, 0, P * lt : P * (lt + 1)],
                                wv_sb[:, 0, 0:32],
                                start=True,
                                stop=True,
                            )
                        else:
                            for e in range(EK):
                                nc.tensor.matmul(
                                    ps[:],
                                    x_chunks[g][:, e, P * lt : P * (lt + 1)],
                                    wv_sb[:, e, :],
                                    start=(e == 0),
                                    stop=(e == EK - 1),
                                )
                        nc.vector.tensor_copy(
                            v_sb[:, t].rearrange("p (h c) -> p h c", h=HEADS_PER_CORE)[
                                :, :, :D
                            ],
                            ps[:].rearrange("p (h c) -> p h c", h=HEADS_PER_CORE),
                        )
                        # cols D:P stay all-ones (memset once, never rewritten)
                    return run

                def outproj_chunk(i, n):
                    def run():
                        po = psP.tile([P, 512], f32, tag="pp", name="ps_out")
                        for p2 in range(2):
                            nc.tensor.matmul(
                                po[:],
                                onorm[:, p2, P * i : P * (i + 1)],
                                wout_sb[:, p2, 512 * n : 512 * (n + 1)],
                                start=(p2 == 0),
                                stop=(p2 == 1),
                            )
                        osb = out_tiles[i]
                        if not skip_ocopy:
                            nc.vector.tensor_copy(
                                osb[:, n * 512 : (n + 1) * 512], po[:]
                            )
                        out_done[i] = out_done.get(i, 0) + 1
                        if out_done[i] == 2 and not skip_out_dma:
                            nc.sync.dma_start(out_d[P * i : P * (i + 1), :], osb[:])
                    return run

                out_tiles = {}
                out_done = {}

                # group 0 prerequisites up front
                load_x_chunk(0)
                if _rep == 0:
                    load_mid_consts()
                fillers = [qk_chunk(f, 0) for f in range(4)]
                fillers += [v_chunk(t) for t in range(4)]

                for g in range(NG):
                    # everything attention(g) needs must be emitted now
                    for fn_ in fillers:
                        fn_()
                    fillers = []
                    if g + 1 < NG:
                        load_x_chunk(g + 1)
                        if g == 0 and _rep == 0:
                            load_wout()
                        fillers += [qk_chunk(f, g + 1) for f in range(4)]
                        fillers += [v_chunk(t) for t in range(4 * (g + 1), 4 * (g + 2))]
                    elif _rep + 1 < repeat and not reuse_x:
                        # prefetch the next rep's first x chunk into the free
                        # rotation buffer while this rep's tail computes
                        x_chunks.pop(0, None)
                        load_x_chunk(0)

                    njt = 4 * g + 4
                    nbatch = njt // 2
                    for hp in range(2):
                        o_ps = {}
                        for a in range(2):
                            o_ps[a] = psO.tile(
                                [P, 512], f32, tag=f"o{a}", name=f"o_ps{a}"
                            )
                        for u in range(nbatch):
                            e_t = {}
                            for a in range(2):
                                s_ps = psS.tile(
                                    [P, 2, 512], f32, tag=f"s{a}", name=f"s_ps{a}"
                                )
                                h = 2 * hp + a
                                for jj in range(2):
                                    jt = 2 * u + jj
                                    # causal: queries < 128*jt never see this
                                    # key tile; skip those columns entirely
                                    cs = max(0, P * (jt - 4 * g))
                                    if skip_scores:
                                        if jj == 0:
                                            nc.tensor.matmul(
                                                s_ps[:, 0, 480:512],
                                                kpad[:, h, 0:P],
                                                qkT[:, hp, 0:32],
                                                start=True,
                                                stop=True,
                                            )
                                        continue
                                    nc.tensor.matmul(
                                        s_ps[:, jj, cs:512],
                                        kpad[:, h, P * jt : P * (jt + 1)],
                                        qkT[
                                            :,
                                            hp,
                                            512 * g + cs : 512 * (g + 1),
                                        ],
                                        start=True,
                                        stop=True,
                                    )
                                e_t[a] = e_p.tile(
                                    [P, 2, 512], bf16, tag=f"e{a}", name=f"e_t{a}"
                                )
                                if skip_exp:
                                    nc.scalar.activation(
                                        e_t[a][:, :, :32],
                                        s_ps[:, :, :32],
                                        AF.Exp,
                                        scale=float(D) ** -0.5,
                                    )
                                elif 2 * u + 1 < 4 * g:
                                    nc.scalar.activation(
                                        e_t[a][:],
                                        s_ps[:],
                                        AF.Exp,
                                        scale=float(D) ** -0.5,
                                    )
                                else:
                                    for jj in range(2):
                                        jt = 2 * u + jj
                                        cs = P * (jt - 4 * g)
                                        nc.scalar.activation(
                                            e_t[a][:, jj, cs:512],
                                            s_ps[:, jj, cs:512],
                                            AF.Exp,
                                            scale=float(D) ** -0.5,
                                        )
                                        nc.vector.tensor_tensor(
                                            e_t[a][:, jj, cs : cs + P],
                                            e_t[a][:, jj, cs : cs + P],
                                            tri_sb[:],
                                            OP.mult,
                                        )
                            # a-outer so each PSUM bank sees its two
                            # accumulating matmuls back-to-back (bank
                            # ping-pong mid-group is expensive on HW)
                            for a in range(2):
                                h = 2 * hp + a
                                for jj in range(2):
                                    jt = 2 * u + jj
                                    cs = P * (jt - 4 * g) if jt >= 4 * g else 0
                                    if skip_o:
                                        if jt == 0 and jj == 0:
                                            nc.tensor.matmul(
                                                o_ps[a][:, 0:32],
                                                v_sb[:, 0, P * h : P * (h + 1)],
                                                e_t[a][:, 0, 0:32],
                                                start=True,
                                                stop=True,
                                            )
                                        continue
                                    nc.tensor.matmul(
                                        o_ps[a][:, cs:512],
                                        v_sb[:, jt, P * h : P * (h + 1)],
                                        e_t[a][:, jj, cs:512],
                                        start=(jt == 0),
                                        stop=(jt == njt - 1),
                                    )
                            if fillers:
                                fillers.pop(0)()
                        # normalization for the pair: sums sit replicated on
                        # o_ps rows 64-127 (ones columns of V), so reciprocal
                        # and multiply run straight on DVE, no broadcast step
                        for a in range(2):
                            bc_sb = bc_p.tile([D, 512], f32, tag="bc", name="bc_sb")
                            nc.vector.reciprocal(bc_sb[:], o_ps[a][D : 2 * D, :])
                            nc.vector.tensor_tensor(
                                onorm[
                                    64 * a : 64 * a + 64, hp, 512 * g : 512 * (g + 1)
                                ],
                                o_ps[a][:D, :],
                                bc_sb[:],
                                OP.mult,
                            )

                    # out-proj chunks for this group become fillers, except the
                    # last group which must run now
                    new_chunks = []
                    for s in range(4):
                        i = 4 * g + s
                        out_tiles[i] = out_p.tile([P, EMB], bf16, tag="osb", name="osb")
                        for n in range(2):
                            new_chunks.append(outproj_chunk(i, n))
                    if g == NG - 1:
                        for fn_ in new_chunks:
                            fn_()
                    else:
                        fillers += new_chunks

    nc.compile()
    return nc


def make_in_maps(x, Wqkv, bqkv, Wout, bout):
    import ml_dtypes

    bf16 = ml_dtypes.bfloat16
    x = np.asarray(x, dtype=np.float32)
    Wqkv = np.asarray(Wqkv, dtype=np.float32).astype(bf16)
    bqkv = np.asarray(bqkv, dtype=np.float32)
    Wout = np.asarray(Wout, dtype=np.float32).astype(bf16)

    trimask = np.triu(np.ones((P, P), dtype=bf16))
    xT = [np.ascontiguousarray(x[b].T).astype(bf16) for b in range(B)]

    in_maps = []
    for c in range(N_CORES):
        b, m = divmod(c, TP)
        q0 = FSH * m
        wqk = np.concatenate(
            [Wqkv[:, q0 : q0 + FSH], Wqkv[:, H * D + q0 : H * D + q0 + FSH]], axis=1
        )
        wv = np.ascontiguousarray(Wqkv[:, 2 * H * D + q0 : 2 * H * D + q0 + FSH])
        wout = np.ascontiguousarray(Wout[q0 : q0 + FSH, :])
        bqk = np.concatenate(
            [bqkv[q0 : q0 + FSH], bqkv[H * D + q0 : H * D + q0 + FSH]]
        ).astype(np.float32)
        in_maps.append(
            {
                "xT": xT[b],
                "wqk": np.ascontiguousarray(wqk),
                "wv": wv,
                "wout": wout,
                "bqk": np.ascontiguousarray(bqk.reshape(4, P).T),
                "trimask": trimask,
            }
        )
    return in_maps


def assemble_output(results, bqkv, Wout, bout):
    bqkv = np.asarray(bqkv, dtype=np.float32)
    Wout = np.asarray(Wout, dtype=np.float32)
    bout = np.asarray(bout, dtype=np.float32)
    # v-bias contribution folded into the output bias (attn rows sum to 1)
    bout_eff = bout + bqkv[2 * H * D :] @ Wout

    out = np.empty((B, T, EMB), dtype=np.float32)
    for b in range(B):
        acc = results[TP * b]["out"].astype(np.float32)
        for m in range(1, TP):
            acc = acc + results[TP * b + m]["out"].astype(np.float32)
        out[b] = acc + bout_eff
    return out


def kernel(x, Wqkv, bqkv, Wout, bout):
    from concourse.bass_utils import run_bass_kernel_spmd

    if "nc" not in _prog_cache:
        _prog_cache["nc"] = _build_program()
    nc = _prog_cache["nc"]

    in_maps = make_in_maps(x, Wqkv, bqkv, Wout, bout)
    res = run_bass_kernel_spmd(nc, in_maps, core_ids=list(range(N_CORES)))
    _prog_cache["last_result"] = res
    return assemble_output(res.results, bqkv, Wout, bout)



# revision 19
# speedup vs baseline: 1.0695x; 1.0695x over previous
"""Causal self-attention Trainium2 kernel.

Reference computation (B=2, T=2048, EMB=1024, H=16 heads, D=64):
    qkv = x @ Wqkv + bqkv ; split q,k,v ; per-head causal softmax attention ;
    out = concat_heads @ Wout + bout

Sharding: 8 cores = data-parallel over batch (2) x tensor-parallel over
heads (4 heads/core).  Each core computes, for its (batch b, head shard m):
  - qkT = (x_b @ Wqk_m)^T  in d-major layout [512, 2048]  (+ bias)
  - v   = x_b @ Wv_m       in t-major layout (v bias folded into bout on host)
  - per head: S^T = k q^T (scale folded into exp), E = exp(S^T) restricted to
    the causal region, O'^T = [v | 1]^T E^T  (ones column yields softmax sums),
    normalize via reciprocal + K=1 matmul partition-broadcast
  - partial out = O_norm^T^T @ Wout_m  -> host sums the 4 TP partials per batch
Data path runs in bfloat16 (fp32 PSUM accumulation; softmax normalization in
fp32r): inputs are downcast host-side, halving DMA traffic, and every matmul
streams at the PE's 1 column/cycle fast path with no fp32r small-N penalty.
Score matmuls for causal-diagonal blocks stream only the unmasked columns.
"""

import sys

sys.path.insert(0, "/opt/trn_rl_repo")

import numpy as np

B, T, EMB = 2, 2048, 1024
H, D = 16, 64
N_CORES = 8
TP = 4  # head shards
HEADS_PER_CORE = H // TP  # 4
FSH = HEADS_PER_CORE * D  # 256 features per shard for each of q,k,v
P = 128
NG = T // 512  # 4 query groups of 512
NT = T // P  # 16 tiles of 128

_prog_cache = {}


def _build_program(
    repeat=1,
    skip_exp=False,
    skip_out_dma=False,
    reuse_x=False,
    skip_scores=False,
    skip_o=False,
    skip_proj=False,
    skip_ocopy=False,
):
    # skip_exp / skip_out_dma / reuse_x are timing-ablation knobs used only by
    # the local bench harness; the graded path always uses the defaults.
    import concourse.mybir as mybir
    import concourse.tile as tile
    from concourse import bacc

    f32 = mybir.dt.float32
    f32r = mybir.dt.float32r
    bf16 = mybir.dt.bfloat16
    AF = mybir.ActivationFunctionType
    OP = mybir.AluOpType

    nc = bacc.Bacc("TRN2", target_bir_lowering=False)

    xT_d = nc.dram_tensor("xT", (EMB, T), bf16, kind="ExternalInput")
    wqk_d = nc.dram_tensor("wqk", (EMB, 2 * FSH), bf16, kind="ExternalInput")
    wv_d = nc.dram_tensor("wv", (EMB, FSH), bf16, kind="ExternalInput")
    wout_d = nc.dram_tensor("wout", (FSH, EMB), bf16, kind="ExternalInput")
    bqk_d = nc.dram_tensor("bqk", (P, 4), f32, kind="ExternalInput")
    tri_d = nc.dram_tensor("trimask", (P, P), bf16, kind="ExternalInput")
    out_d = nc.dram_tensor("out", (T, EMB), bf16, kind="ExternalOutput")

    EK = EMB // P  # 8 contraction chunks
    xT_r = xT_d.rearrange("(o p) t -> p o t", p=P)

    with nc.allow_low_precision(
        reason="float32r tiles feed the PE fast path; fp32 accumulation in PSUM"
    ), tile.TileContext(nc) as tc:
        with (
            tc.tile_pool(name="consts", bufs=1) as consts,
            tc.tile_pool(name="qkt", bufs=1) as qkt_p,
            tc.tile_pool(name="vp", bufs=1) as v_p,
            tc.tile_pool(name="onorm", bufs=1) as onorm_p,
            tc.tile_pool(name="xp", bufs=4 if reuse_x else 2) as xp,
            tc.tile_pool(name="ep", bufs=3) as e_p,
            tc.tile_pool(name="bc", bufs=2) as bc_p,
            tc.tile_pool(name="outsb", bufs=5) as out_p,
            tc.tile_pool(name="psP", bufs=2, space="PSUM") as psP,
            tc.tile_pool(name="psS", bufs=1, space="PSUM") as psS,
            tc.tile_pool(name="psO", bufs=1, space="PSUM") as psO,
        ):
            # Startup-critical ordering: bias (tiny) + wqk f-tile 0 first, then
            # x chunk 0 is issued in the rep loop, then remaining weights are
            # staged so each arrives just before its first consumer.
            wqk_r = wqk_d.rearrange("(o p) f -> p o f", p=P)
            # one tile per f-slice so the first consumer only waits on its own DMA
            wqk_fs = [
                consts.tile([P, EK, P], bf16, name=f"wqk_f{f}") for f in range(4)
            ]
            wv_sb = consts.tile([P, EK, FSH], bf16)
            bqk_sb = consts.tile([P, 4], f32)
            tri_sb = consts.tile([P, P], bf16)
            wout_sb = consts.tile([P, 2, EMB], bf16)

            nc.sync.dma_start(bqk_sb[:], bqk_d[:])
            nc.sync.dma_start(wqk_fs[0][:], wqk_r[:, :, 0:P])

            def load_mid_consts():
                nc.sync.dma_start(wqk_fs[1][:], wqk_r[:, :, P : 2 * P])
                nc.sync.dma_start(wv_sb[:], wv_d.rearrange("(o p) f -> p o f", p=P))
                nc.sync.dma_start(wqk_fs[2][:], wqk_r[:, :, 2 * P : 3 * P])
                nc.sync.dma_start(wqk_fs[3][:], wqk_r[:, :, 3 * P : 4 * P])
                nc.sync.dma_start(tri_sb[:], tri_d[:])

            def load_wout():
                nc.sync.dma_start(
                    wout_sb[:], wout_d.rearrange("(o p) f -> p o f", p=P)
                )

            qkT = qkt_p.tile([P, 2, T], bf16)
            # per-head k tiles, zero-padded on the opposite 64-row half so the
            # score matmul can contract over all 128 partitions (the zero rows
            # cancel the other head packed in the shared q tile)
            kpad = qkt_p.tile([P, HEADS_PER_CORE, T], bf16)
            nc.vector.memset(kpad[:], 0.0)
            # per-head V blocks padded to 128 columns: cols 0-63 hold V, cols
            # 64-127 are all-ones so the O matmul (M=128, full-speed) emits the
            # softmax sums replicated on output partitions 64-127
            v_sb = v_p.tile([P, NT, HEADS_PER_CORE * P], bf16)
            v_ones_view = v_sb.rearrange("p t (h c) -> p t h c", c=P)[:, :, :, D:P]
            nc.vector.memset(v_ones_view, 1.0)

            x_chunks = {}

            for _rep in range(repeat):
                onorm = onorm_p.tile([P, 2, T], bf16)

                # ---------- chunked x load + deferred PE work ----------

                def load_x_chunk(g):
                    if reuse_x and g in x_chunks:
                        return
                    x_chunks[g] = xp.tile([P, EK, 512], bf16, name="xchunk")
                    nc.sync.dma_start(
                        x_chunks[g][:], xT_r[:, :, 512 * g : 512 * (g + 1)]
                    )

                def qk_chunk(f, g):
                    def run():
                        ps = psP.tile([P, 512], f32, tag="pp", name="ps_qk")
                        if skip_proj:
                            nc.tensor.matmul(
                                ps[:, 0:32],
                                wqk_fs[f][:, 0, :],
                                x_chunks[g][:, 0, 0:32],
                                start=True,
                                stop=True,
                            )
                        else:
                            for e in range(EK):
                                nc.tensor.matmul(
                                    ps[:],
                                    wqk_fs[f][:, e, :],
                                    x_chunks[g][:, e, :],
                                    start=(e == 0),
                                    stop=(e == EK - 1),
                                )
                        if f < 2:
                            nc.vector.tensor_scalar_add(
                                qkT[:, f, 512 * g : 512 * (g + 1)],
                                ps[:],
                                bqk_sb[:, f : f + 1],
                            )
                        else:
                            # k features of heads 2(f-2) / 2(f-2)+1 land on
                            # partition halves 0-63 / 64-127 — exactly the
                            # zero-padded halves their kpad slices use
                            h0 = 2 * (f - 2)
                            nc.vector.tensor_scalar_add(
                                kpad[0:64, h0, 512 * g : 512 * (g + 1)],
                                ps[0:64, :],
                                bqk_sb[0:64, f : f + 1],
                            )
                            nc.vector.tensor_scalar_add(
                                kpad[64:128, h0 + 1, 512 * g : 512 * (g + 1)],
                                ps[64:128, :],
                                bqk_sb[64:128, f : f + 1],
                            )
                    return run

                def v_chunk(t):
                    def run():
                        g, lt = divmod(t, 4)
                        ps = psP.tile([P, FSH], f32, tag="pp", name="ps_v")
                        if skip_proj:
                            nc.tensor.matmul(
                                ps[:, 0:32],
                                x_chunks[g][:, 0, P * lt : P * (lt + 1)],
                                wv_sb[:, 0, 0:32],
                                start=True,
                                stop=True,
                            )
                        else:
                            for e in range(EK):
                                nc.tensor.matmul(
                                    ps[:],
                                    x_chunks[g][:, e, P * lt : P * (lt + 1)],
                                    wv_sb[:, e, :],
                                    start=(e == 0),
                                    stop=(e == EK - 1),
                                )
                        nc.vector.tensor_copy(
                            v_sb[:, t].rearrange("p (h c) -> p h c", h=HEADS_PER_CORE)[
                                :, :, :D
                            ],
                            ps[:].rearrange("p (h c) -> p h c", h=HEADS_PER_CORE),
                        )
                        # cols D:P stay all-ones (memset once, never rewritten)
                    return run

                def outproj_chunk(i, n):
                    def run():
                        po = psP.tile([P, 512], f32, tag="pp", name="ps_out")
                        for p2 in range(2):
                            nc.tensor.matmul(
                                po[:],
                                onorm[:, p2, P * i : P * (i + 1)],
                                wout_sb[:, p2, 512 * n : 512 * (n + 1)],
                                start=(p2 == 0),
                                stop=(p2 == 1),
                            )
                        osb = out_tiles[i]
                        if not skip_ocopy:
                            nc.vector.tensor_copy(
                                osb[:, n * 512 : (n + 1) * 512], po[:]
                            )
                        out_done[i] = out_done.get(i, 0) + 1
                        if out_done[i] == 2 and not skip_out_dma:
                            nc.sync.dma_start(out_d[P * i : P * (i + 1), :], osb[:])
                    return run

                out_tiles = {}
                out_done = {}

                # group 0 prerequisites up front
                load_x_chunk(0)
                if _rep == 0:
                    load_mid_consts()
                fillers = [qk_chunk(f, 0) for f in range(4)]
                fillers += [v_chunk(t) for t in range(4)]

                for g in range(NG):
                    # everything attention(g) needs must be emitted now
                    for fn_ in fillers:
                        fn_()
                    fillers = []
                    if g + 1 < NG:
                        load_x_chunk(g + 1)
                        if g == 0 and _rep == 0:
                            load_wout()
                        fillers += [qk_chunk(f, g + 1) for f in range(4)]
                        fillers += [v_chunk(t) for t in range(4 * (g + 1), 4 * (g + 2))]


                    njt = 4 * g + 4
                    nbatch = njt // 2
                    for hp in range(2):
                        o_ps = {}
                        for a in range(2):
                            o_ps[a] = psO.tile(
                                [P, 512], f32, tag=f"o{a}", name=f"o_ps{a}"
                            )
                        for u in range(nbatch):
                            e_t = {}
                            for a in range(2):
                                s_ps = psS.tile(
                                    [P, 2, 512], f32, tag=f"s{a}", name=f"s_ps{a}"
                                )
                                h = 2 * hp + a
                                for jj in range(2):
                                    jt = 2 * u + jj
                                    # causal: queries < 128*jt never see this
                                    # key tile; skip those columns entirely
                                    cs = max(0, P * (jt - 4 * g))
                                    if skip_scores:
                                        if jj == 0:
                                            nc.tensor.matmul(
                                                s_ps[:, 0, 480:512],
                                                kpad[:, h, 0:P],
                                                qkT[:, hp, 0:32],
                                                start=True,
                                                stop=True,
                                            )
                                        continue
                                    nc.tensor.matmul(
                                        s_ps[:, jj, cs:512],
                                        kpad[:, h, P * jt : P * (jt + 1)],
                                        qkT[
                                            :,
                                            hp,
                                            512 * g + cs : 512 * (g + 1),
                                        ],
                                        start=True,
                                        stop=True,
                                    )
                                e_t[a] = e_p.tile(
                                    [P, 2, 512], bf16, tag=f"e{a}", name=f"e_t{a}"
                                )
                                if skip_exp:
                                    nc.scalar.activation(
                                        e_t[a][:, :, :32],
                                        s_ps[:, :, :32],
                                        AF.Exp,
                                        scale=float(D) ** -0.5,
                                    )
                                elif 2 * u + 1 < 4 * g:
                                    nc.scalar.activation(
                                        e_t[a][:],
                                        s_ps[:],
                                        AF.Exp,
                                        scale=float(D) ** -0.5,
                                    )
                                else:
                                    for jj in range(2):
                                        jt = 2 * u + jj
                                        cs = P * (jt - 4 * g)
                                        nc.scalar.activation(
                                            e_t[a][:, jj, cs:512],
                                            s_ps[:, jj, cs:512],
                                            AF.Exp,
                                            scale=float(D) ** -0.5,
                                        )
                                        nc.vector.tensor_tensor(
                                            e_t[a][:, jj, cs : cs + P],
                                            e_t[a][:, jj, cs : cs + P],
                                            tri_sb[:],
                                            OP.mult,
                                        )
                            # a-outer so each PSUM bank sees its two
                            # accumulating matmuls back-to-back (bank
                            # ping-pong mid-group is expensive on HW)
                            for a in range(2):
                                h = 2 * hp + a
                                for jj in range(2):
                                    jt = 2 * u + jj
                                    cs = P * (jt - 4 * g) if jt >= 4 * g else 0
                                    if skip_o:
                                        if jt == 0 and jj == 0:
                                            nc.tensor.matmul(
                                                o_ps[a][:, 0:32],
                                                v_sb[:, 0, P * h : P * (h + 1)],
                                                e_t[a][:, 0, 0:32],
                                                start=True,
                                                stop=True,
                                            )
                                        continue
                                    nc.tensor.matmul(
                                        o_ps[a][:, cs:512],
                                        v_sb[:, jt, P * h : P * (h + 1)],
                                        e_t[a][:, jj, cs:512],
                                        start=(jt == 0),
                                        stop=(jt == njt - 1),
                                    )
                            if fillers:
                                fillers.pop(0)()
                        # normalization for the pair: sums sit replicated on
                        # o_ps rows 64-127 (ones columns of V), so reciprocal
                        # and multiply run straight on DVE, no broadcast step
                        for a in range(2):
                            bc_sb = bc_p.tile([D, 512], f32, tag="bc", name="bc_sb")
                            nc.vector.reciprocal(bc_sb[:], o_ps[a][D : 2 * D, :])
                            nc.vector.tensor_tensor(
                                onorm[
                                    64 * a : 64 * a + 64, hp, 512 * g : 512 * (g + 1)
                                ],
                                o_ps[a][:D, :],
                                bc_sb[:],
                                OP.mult,
                            )

                    # out-proj chunks for this group become fillers, except the
                    # last group which must run now
                    new_chunks = []
                    for s in range(4):
                        i = 4 * g + s
                        out_tiles[i] = out_p.tile([P, EMB], bf16, tag="osb", name="osb")
                        for n in range(2):
                            new_chunks.append(outproj_chunk(i, n))
                    if g == NG - 1:
                        for fn_ in new_chunks:
                            fn_()
                    else:
                        fillers += new_chunks

    nc.compile()
    return nc


def make_in_maps(x, Wqkv, bqkv, Wout, bout):
    import ml_dtypes

    bf16 = ml_dtypes.bfloat16
    x = np.asarray(x, dtype=np.float32)
    Wqkv = np.asarray(Wqkv, dtype=np.float32).astype(bf16)
    bqkv = np.asarray(bqkv, dtype=np.float32)
    Wout = np.asarray(Wout, dtype=np.float32).astype(bf16)

    trimask = np.triu(np.ones((P, P), dtype=bf16))
    xT = [np.ascontiguousarray(x[b].T).astype(bf16) for b in range(B)]

    in_maps = []
    for c in range(N_CORES):
        b, m = divmod(c, TP)
        q0 = FSH * m
        wqk = np.concatenate(
            [Wqkv[:, q0 : q0 + FSH], Wqkv[:, H * D + q0 : H * D + q0 + FSH]], axis=1
        )
        wv = np.ascontiguousarray(Wqkv[:, 2 * H * D + q0 : 2 * H * D + q0 + FSH])
        wout = np.ascontiguousarray(Wout[q0 : q0 + FSH, :])
        bqk = np.concatenate(
            [bqkv[q0 : q0 + FSH], bqkv[H * D + q0 : H * D + q0 + FSH]]
        ).astype(np.float32)
        in_maps.append(
            {
                "xT": xT[b],
                "wqk": np.ascontiguousarray(wqk),
                "wv": wv,
                "wout": wout,
                "bqk": np.ascontiguousarray(bqk.reshape(4, P).T),
                "trimask": trimask,
            }
        )
    return in_maps


def assemble_output(results, bqkv, Wout, bout):
    bqkv = np.asarray(bqkv, dtype=np.float32)
    Wout = np.asarray(Wout, dtype=np.float32)
    bout = np.asarray(bout, dtype=np.float32)
    # v-bias contribution folded into the output bias (attn rows sum to 1)
    bout_eff = bout + bqkv[2 * H * D :] @ Wout

    out = np.empty((B, T, EMB), dtype=np.float32)
    for b in range(B):
        acc = results[TP * b]["out"].astype(np.float32)
        for m in range(1, TP):
            acc = acc + results[TP * b + m]["out"].astype(np.float32)
        out[b] = acc + bout_eff
    return out


def kernel(x, Wqkv, bqkv, Wout, bout):
    from concourse.bass_utils import run_bass_kernel_spmd

    if "nc" not in _prog_cache:
        _prog_cache["nc"] = _build_program()
    nc = _prog_cache["nc"]

    in_maps = make_in_maps(x, Wqkv, bqkv, Wout, bout)
    res = run_bass_kernel_spmd(nc, in_maps, core_ids=list(range(N_CORES)))
    _prog_cache["last_result"] = res
    return assemble_output(res.results, bqkv, Wout, bout)



# revision 20
# speedup vs baseline: 1.0943x; 1.0232x over previous
"""Causal self-attention Trainium2 kernel.

Reference computation (B=2, T=2048, EMB=1024, H=16 heads, D=64):
    qkv = x @ Wqkv + bqkv ; split q,k,v ; per-head causal softmax attention ;
    out = concat_heads @ Wout + bout

Sharding: 8 cores = data-parallel over batch (2) x tensor-parallel over
heads (4 heads/core).  Each core computes, for its (batch b, head shard m):
  - q^T = (x_b @ Wq_m)^T  (2 heads packed per 128 partitions, + bias)
  - kpad: per-head (x_b @ Wk_m)^T zero-padded on the opposite 64-partition
    half, so score matmuls contract over the full 128 partitions (the zero
    rows cancel the other head packed in the shared q tile); K=128 matmuls
    run ~3x faster than K=64 on TRN2
  - v = x_b @ Wv_m in t-major layout, each head's block padded to 128 columns
    with 64 all-ones columns: the M=128 O matmul then emits softmax sums
    replicated on output partitions 64-127, so normalization is a plain DVE
    reciprocal + multiply (no partition broadcast needed)
  - per head: S^T = kpad^T q (scale folded into exp), E = exp(S^T) restricted
    to the causal region (diagonal blocks stream only unmasked columns),
    O'^T accumulated a-outer so each PSUM bank sees consecutive matmuls
  - partial out = O_norm^T^T @ Wout_m -> host sums the 4 TP partials per batch
Data path is bfloat16 (fp32 PSUM accumulation and softmax normalization);
inputs are downcast host-side, halving DMA traffic.
"""

import sys

sys.path.insert(0, "/opt/trn_rl_repo")

import numpy as np

B, T, EMB = 2, 2048, 1024
H, D = 16, 64
N_CORES = 8
TP = 4  # head shards
HEADS_PER_CORE = H // TP  # 4
FSH = HEADS_PER_CORE * D  # 256 features per shard for each of q,k,v
P = 128
NG = T // 512  # 4 query groups of 512
NT = T // P  # 16 tiles of 128

_prog_cache = {}


def _build_program(
    repeat=1,
    skip_exp=False,
    skip_out_dma=False,
    reuse_x=False,
    skip_scores=False,
    skip_o=False,
    skip_proj=False,
    skip_ocopy=False,
):
    # skip_exp / skip_out_dma / reuse_x are timing-ablation knobs used only by
    # the local bench harness; the graded path always uses the defaults.
    import concourse.mybir as mybir
    import concourse.tile as tile
    from concourse import bacc

    f32 = mybir.dt.float32
    f32r = mybir.dt.float32r
    bf16 = mybir.dt.bfloat16
    AF = mybir.ActivationFunctionType
    OP = mybir.AluOpType

    nc = bacc.Bacc("TRN2", target_bir_lowering=False)

    xT_d = nc.dram_tensor("xT", (EMB, T), bf16, kind="ExternalInput")
    wqk_d = nc.dram_tensor("wqk", (EMB, 2 * FSH), bf16, kind="ExternalInput")
    wv_d = nc.dram_tensor("wv", (EMB, FSH), bf16, kind="ExternalInput")
    wout_d = nc.dram_tensor("wout", (FSH, EMB), bf16, kind="ExternalInput")
    bqk_d = nc.dram_tensor("bqk", (P, 4), f32, kind="ExternalInput")
    tri_d = nc.dram_tensor("trimask", (P, P), bf16, kind="ExternalInput")
    out_d = nc.dram_tensor("out", (T, EMB), f32, kind="ExternalOutput")

    EK = EMB // P  # 8 contraction chunks
    xT_r = xT_d.rearrange("(o p) t -> p o t", p=P)

    with nc.allow_low_precision(
        reason="float32r tiles feed the PE fast path; fp32 accumulation in PSUM"
    ), tile.TileContext(nc) as tc:
        with (
            tc.tile_pool(name="consts", bufs=1) as consts,
            tc.tile_pool(name="qkt", bufs=1) as qkt_p,
            tc.tile_pool(name="vp", bufs=1) as v_p,
            tc.tile_pool(name="onorm", bufs=1) as onorm_p,
            tc.tile_pool(name="xp", bufs=4 if reuse_x else 2) as xp,
            tc.tile_pool(name="ep", bufs=3) as e_p,
            tc.tile_pool(name="bc", bufs=2) as bc_p,
            tc.tile_pool(name="outsb", bufs=5) as out_p,
            tc.tile_pool(name="psP", bufs=2, space="PSUM") as psP,
            tc.tile_pool(name="psS", bufs=1, space="PSUM") as psS,
            tc.tile_pool(name="psO", bufs=1, space="PSUM") as psO,
        ):
            # Startup-critical ordering: bias (tiny) + wqk f-tile 0 first, then
            # x chunk 0 is issued in the rep loop, then remaining weights are
            # staged so each arrives just before its first consumer.
            wqk_r = wqk_d.rearrange("(o p) f -> p o f", p=P)
            # one tile per f-slice so the first consumer only waits on its own DMA
            wqk_fs = [
                consts.tile([P, EK, P], bf16, name=f"wqk_f{f}") for f in range(4)
            ]
            wv_sb = consts.tile([P, EK, FSH], bf16)
            bqk_sb = consts.tile([P, 4], f32)
            tri_sb = consts.tile([P, P], bf16)
            wout_sb = consts.tile([P, 2, EMB], bf16)

            nc.sync.dma_start(bqk_sb[:], bqk_d[:])
            nc.sync.dma_start(wqk_fs[0][:], wqk_r[:, :, 0:P])

            def load_mid_consts():
                nc.sync.dma_start(wqk_fs[1][:], wqk_r[:, :, P : 2 * P])
                nc.sync.dma_start(wv_sb[:], wv_d.rearrange("(o p) f -> p o f", p=P))
                nc.sync.dma_start(wqk_fs[2][:], wqk_r[:, :, 2 * P : 3 * P])
                nc.sync.dma_start(wqk_fs[3][:], wqk_r[:, :, 3 * P : 4 * P])
                nc.sync.dma_start(tri_sb[:], tri_d[:])

            def load_wout():
                nc.sync.dma_start(
                    wout_sb[:], wout_d.rearrange("(o p) f -> p o f", p=P)
                )

            qkT = qkt_p.tile([P, 2, T], bf16)
            # per-head k tiles, zero-padded on the opposite 64-row half so the
            # score matmul can contract over all 128 partitions (the zero rows
            # cancel the other head packed in the shared q tile)
            kpad = qkt_p.tile([P, HEADS_PER_CORE, T], bf16)
            nc.vector.memset(kpad[:], 0.0)
            # per-head V blocks padded to 128 columns: cols 0-63 hold V, cols
            # 64-127 are all-ones so the O matmul (M=128, full-speed) emits the
            # softmax sums replicated on output partitions 64-127
            v_sb = v_p.tile([P, NT, HEADS_PER_CORE * P], bf16)
            v_ones_view = v_sb.rearrange("p t (h c) -> p t h c", c=P)[:, :, :, D:P]
            nc.vector.memset(v_ones_view, 1.0)

            x_chunks = {}

            for _rep in range(repeat):
                onorm = onorm_p.tile([P, 2, T], bf16)

                # ---------- chunked x load + deferred PE work ----------

                def load_x_chunk(g):
                    if reuse_x and g in x_chunks:
                        return
                    x_chunks[g] = xp.tile([P, EK, 512], bf16, name="xchunk")
                    nc.sync.dma_start(
                        x_chunks[g][:], xT_r[:, :, 512 * g : 512 * (g + 1)]
                    )

                def qk_chunk(f, g):
                    def run():
                        ps = psP.tile([P, 512], f32, tag="pp", name="ps_qk")
                        if skip_proj:
                            nc.tensor.matmul(
                                ps[:, 0:32],
                                wqk_fs[f][:, 0, :],
                                x_chunks[g][:, 0, 0:32],
                                start=True,
                                stop=True,
                            )
                        else:
                            for e in range(EK):
                                nc.tensor.matmul(
                                    ps[:],
                                    wqk_fs[f][:, e, :],
                                    x_chunks[g][:, e, :],
                                    start=(e == 0),
                                    stop=(e == EK - 1),
                                )
                        if f < 2:
                            nc.vector.tensor_scalar_add(
                                qkT[:, f, 512 * g : 512 * (g + 1)],
                                ps[:],
                                bqk_sb[:, f : f + 1],
                            )
                        else:
                            # k features of heads 2(f-2) / 2(f-2)+1 land on
                            # partition halves 0-63 / 64-127 — exactly the
                            # zero-padded halves their kpad slices use
                            h0 = 2 * (f - 2)
                            nc.vector.tensor_scalar_add(
                                kpad[0:64, h0, 512 * g : 512 * (g + 1)],
                                ps[0:64, :],
                                bqk_sb[0:64, f : f + 1],
                            )
                            nc.vector.tensor_scalar_add(
                                kpad[64:128, h0 + 1, 512 * g : 512 * (g + 1)],
                                ps[64:128, :],
                                bqk_sb[64:128, f : f + 1],
                            )
                    return run

                def v_chunk(t):
                    def run():
                        g, lt = divmod(t, 4)
                        ps = psP.tile([P, FSH], f32, tag="pp", name="ps_v")
                        if skip_proj:
                            nc.tensor.matmul(
                                ps[:, 0:32],
                                x_chunks[g][:, 0, P * lt : P * (lt + 1)],
                                wv_sb[:, 0, 0:32],
                                start=True,
                                stop=True,
                            )
                        else:
                            for e in range(EK):
                                nc.tensor.matmul(
                                    ps[:],
                                    x_chunks[g][:, e, P * lt : P * (lt + 1)],
                                    wv_sb[:, e, :],
                                    start=(e == 0),
                                    stop=(e == EK - 1),
                                )
                        nc.vector.tensor_copy(
                            v_sb[:, t].rearrange("p (h c) -> p h c", h=HEADS_PER_CORE)[
                                :, :, :D
                            ],
                            ps[:].rearrange("p (h c) -> p h c", h=HEADS_PER_CORE),
                        )
                        # cols D:P stay all-ones (memset once, never rewritten)
                    return run

                def outproj_chunk(i, n):
                    def run():
                        po = psP.tile([P, 512], f32, tag="pp", name="ps_out")
                        for p2 in range(2):
                            nc.tensor.matmul(
                                po[:],
                                onorm[:, p2, P * i : P * (i + 1)],
                                wout_sb[:, p2, 512 * n : 512 * (n + 1)],
                                start=(p2 == 0),
                                stop=(p2 == 1),
                            )
                        osb = out_tiles[i]
                        if not skip_ocopy:
                            nc.vector.tensor_copy(
                                osb[:, n * 512 : (n + 1) * 512], po[:]
                            )
                        out_done[i] = out_done.get(i, 0) + 1
                        if out_done[i] == 2 and not skip_out_dma:
                            nc.sync.dma_start(out_d[P * i : P * (i + 1), :], osb[:])
                    return run

                out_tiles = {}
                out_done = {}

                # group 0 prerequisites up front
                load_x_chunk(0)
                if _rep == 0:
                    load_mid_consts()
                fillers = [qk_chunk(f, 0) for f in range(4)]
                fillers += [v_chunk(t) for t in range(4)]

                for g in range(NG):
                    # everything attention(g) needs must be emitted now
                    for fn_ in fillers:
                        fn_()
                    fillers = []
                    if g + 1 < NG:
                        load_x_chunk(g + 1)
                        if g == 0 and _rep == 0:
                            load_wout()
                        fillers += [qk_chunk(f, g + 1) for f in range(4)]
                        fillers += [v_chunk(t) for t in range(4 * (g + 1), 4 * (g + 2))]


                    njt = 4 * g + 4
                    nbatch = njt // 2
                    for hp in range(2):
                        o_ps = {}
                        for a in range(2):
                            o_ps[a] = psO.tile(
                                [P, 512], f32, tag=f"o{a}", name=f"o_ps{a}"
                            )
                        for u in range(nbatch):
                            e_t = {}
                            for a in range(2):
                                s_ps = psS.tile(
                                    [P, 2, 512], f32, tag=f"s{a}", name=f"s_ps{a}"
                                )
                                h = 2 * hp + a
                                for jj in range(2):
                                    jt = 2 * u + jj
                                    # causal: queries < 128*jt never see this
                                    # key tile; skip those columns entirely
                                    cs = max(0, P * (jt - 4 * g))
                                    if skip_scores:
                                        if jj == 0:
                                            nc.tensor.matmul(
                                                s_ps[:, 0, 480:512],
                                                kpad[:, h, 0:P],
                                                qkT[:, hp, 0:32],
                                                start=True,
                                                stop=True,
                                            )
                                        continue
                                    nc.tensor.matmul(
                                        s_ps[:, jj, cs:512],
                                        kpad[:, h, P * jt : P * (jt + 1)],
                                        qkT[
                                            :,
                                            hp,
                                            512 * g + cs : 512 * (g + 1),
                                        ],
                                        start=True,
                                        stop=True,
                                    )
                                e_t[a] = e_p.tile(
                                    [P, 2, 512], bf16, tag=f"e{a}", name=f"e_t{a}"
                                )
                                if skip_exp:
                                    nc.scalar.activation(
                                        e_t[a][:, :, :32],
                                        s_ps[:, :, :32],
                                        AF.Exp,
                                        scale=float(D) ** -0.5,
                                    )
                                elif 2 * u + 1 < 4 * g:
                                    nc.scalar.activation(
                                        e_t[a][:],
                                        s_ps[:],
                                        AF.Exp,
                                        scale=float(D) ** -0.5,
                                    )
                                else:
                                    for jj in range(2):
                                        jt = 2 * u + jj
                                        cs = P * (jt - 4 * g)
                                        nc.scalar.activation(
                                            e_t[a][:, jj, cs:512],
                                            s_ps[:, jj, cs:512],
                                            AF.Exp,
                                            scale=float(D) ** -0.5,
                                        )
                                        nc.vector.tensor_tensor(
                                            e_t[a][:, jj, cs : cs + P],
                                            e_t[a][:, jj, cs : cs + P],
                                            tri_sb[:],
                                            OP.mult,
                                        )
                            # a-outer so each PSUM bank sees its two
                            # accumulating matmuls back-to-back (bank
                            # ping-pong mid-group is expensive on HW)
                            for a in range(2):
                                h = 2 * hp + a
                                for jj in range(2):
                                    jt = 2 * u + jj
                                    cs = P * (jt - 4 * g) if jt >= 4 * g else 0
                                    if skip_o:
                                        if jt == 0 and jj == 0:
                                            nc.tensor.matmul(
                                                o_ps[a][:, 0:32],
                                                v_sb[:, 0, P * h : P * (h + 1)],
                                                e_t[a][:, 0, 0:32],
                                                start=True,
                                                stop=True,
                                            )
                                        continue
                                    nc.tensor.matmul(
                                        o_ps[a][:, cs:512],
                                        v_sb[:, jt, P * h : P * (h + 1)],
                                        e_t[a][:, jj, cs:512],
                                        start=(jt == 0),
                                        stop=(jt == njt - 1),
                                    )
                            if fillers:
                                fillers.pop(0)()
                        # normalization for the pair: sums sit replicated on
                        # o_ps rows 64-127 (ones columns of V), so reciprocal
                        # and multiply run straight on DVE, no broadcast step
                        for a in range(2):
                            bc_sb = bc_p.tile([D, 512], f32, tag="bc", name="bc_sb")
                            nc.vector.reciprocal(bc_sb[:], o_ps[a][D : 2 * D, :])
                            nc.vector.tensor_tensor(
                                onorm[
                                    64 * a : 64 * a + 64, hp, 512 * g : 512 * (g + 1)
                                ],
                                o_ps[a][:D, :],
                                bc_sb[:],
                                OP.mult,
                            )

                    # out-proj chunks for this group become fillers, except the
                    # last group which must run now
                    new_chunks = []
                    for s in range(4):
                        i = 4 * g + s
                        out_tiles[i] = out_p.tile([P, EMB], f32, tag="osb", name="osb")
                        for n in range(2):
                            new_chunks.append(outproj_chunk(i, n))
                    if g == NG - 1:
                        for fn_ in new_chunks:
                            fn_()
                    else:
                        fillers += new_chunks

    nc.compile()
    return nc


def make_in_maps(x, Wqkv, bqkv, Wout, bout):
    import ml_dtypes

    bf16 = ml_dtypes.bfloat16
    x = np.asarray(x, dtype=np.float32)
    Wqkv = np.asarray(Wqkv, dtype=np.float32).astype(bf16)
    bqkv = np.asarray(bqkv, dtype=np.float32)
    Wout = np.asarray(Wout, dtype=np.float32).astype(bf16)

    trimask = np.triu(np.ones((P, P), dtype=bf16))
    xT = [np.ascontiguousarray(x[b].T).astype(bf16) for b in range(B)]

    in_maps = []
    for c in range(N_CORES):
        b, m = divmod(c, TP)
        q0 = FSH * m
        wqk = np.concatenate(
            [Wqkv[:, q0 : q0 + FSH], Wqkv[:, H * D + q0 : H * D + q0 + FSH]], axis=1
        )
        wv = np.ascontiguousarray(Wqkv[:, 2 * H * D + q0 : 2 * H * D + q0 + FSH])
        wout = np.ascontiguousarray(Wout[q0 : q0 + FSH, :])
        bqk = np.concatenate(
            [bqkv[q0 : q0 + FSH], bqkv[H * D + q0 : H * D + q0 + FSH]]
        ).astype(np.float32)
        in_maps.append(
            {
                "xT": xT[b],
                "wqk": np.ascontiguousarray(wqk),
                "wv": wv,
                "wout": wout,
                "bqk": np.ascontiguousarray(bqk.reshape(4, P).T),
                "trimask": trimask,
            }
        )
    return in_maps


def assemble_output(results, bqkv, Wout, bout):
    bqkv = np.asarray(bqkv, dtype=np.float32)
    Wout = np.asarray(Wout, dtype=np.float32)
    bout = np.asarray(bout, dtype=np.float32)
    # v-bias contribution folded into the output bias (attn rows sum to 1)
    bout_eff = bout + bqkv[2 * H * D :] @ Wout

    out = np.empty((B, T, EMB), dtype=np.float32)
    for b in range(B):
        acc = results[TP * b]["out"].astype(np.float32)
        for m in range(1, TP):
            acc = acc + results[TP * b + m]["out"].astype(np.float32)
        out[b] = acc + bout_eff
    return out


def kernel(x, Wqkv, bqkv, Wout, bout):
    from concourse.bass_utils import run_bass_kernel_spmd

    if "nc" not in _prog_cache:
        _prog_cache["nc"] = _build_program()
    nc = _prog_cache["nc"]

    in_maps = make_in_maps(x, Wqkv, bqkv, Wout, bout)
    res = run_bass_kernel_spmd(nc, in_maps, core_ids=list(range(N_CORES)))
    _prog_cache["last_result"] = res
    return assemble_output(res.results, bqkv, Wout, bout)

